# revision 10
# baseline (speedup 1.0000x reference)
"""Causal single-head attention (B=4, T=2048, D=1024, fp32) on 8 TRN2 NeuronCores.

Sharding: 2 cores per batch. Within a pair, keys/values are split by
interleaved 128-token tiles (core parity p takes s-tiles t with t%2==p), which
makes the program perfectly uniform across cores (SPMD): for every 512-wide
query chunk i, each core processes exactly 2i+2 local key tiles. Each core
computes an unnormalized partial attention output plus softmax denominators
for ALL queries of its batch; the host merges the two partials per batch
(add, then divide) while unsharding.

All matmuls run as fp32r (TF32-like, ~1.5e-4 rel err, full PE rate at N>=256).
Softmax is computed without max-subtraction: logits = scores/32 are bounded by
~|q||k|/32 ~ +-8 for this distribution, far from fp32 exp overflow.
"""
import numpy as np

B, T, D = 4, 2048, 1024
P = 128
NK = D // P          # 8 contraction tiles
QC = T // 512        # 4 query chunks of 512
NEG = -1e30
SCALE = 1.0 / 32.0   # 1/sqrt(D)

_prog = None
_last_in_maps = None


def _build_program():
    import concourse.bacc as bacc
    import concourse.mybir as mybir
    import concourse.tile as tile

    f32 = mybir.dt.float32
    f32r = mybir.dt.float32r

    nc = bacc.Bacc()
    xt_d = nc.declare_dram_parameter("xt", [D, T], f32r, isOutput=False)
    xtl_d = nc.declare_dram_parameter("xtl", [D, T // 2], f32r, isOutput=False)
    wq_d = nc.declare_dram_parameter("wq", [D, D], f32r, isOutput=False)
    wk_d = nc.declare_dram_parameter("wk", [D, D], f32r, isOutput=False)
    wv_d = nc.declare_dram_parameter("wv", [D, D], f32r, isOutput=False)
    mask_d = nc.declare_dram_parameter("masks", [2, P, 512], f32, isOutput=False)
    ones_d = nc.declare_dram_parameter("ones", [P, 2], f32r, isOutput=False)
    part_d = nc.declare_dram_parameter("part", [T, D + 1], f32, isOutput=True)

    with tile.TileContext(nc) as tc:
        with tc.tile_pool(name="sbuf", bufs=1) as pool, \
             tc.tile_pool(name="psum", bufs=1, space="PSUM") as psum:

            # ---- long-lived tiles ----
            wq_t = pool.tile([P, NK, D], f32r, tag="wq")       # Wq, pinned
            kt_sb = pool.tile([P, NK, T // 2], f32r, tag="kt")  # K^T, local s
            v_sb = pool.tile([P, NK, D], f32r, tag="v")         # V, local s tiles
            mask_t = pool.tile([P, 2, 512], f32, tag="mask")
            ones_t = pool.tile([P, 2], f32r, tag="ones")

            def stage_w(src, h):
                """load [128, NK, 512] = src[:, 512h:512h+512] by k-tiles,
                two DMAs per k-tile so the first consumer waits ~half as long"""
                t = pool.tile([P, NK, 512], f32r, tag="stage", bufs=4)
                c0 = 512 * h
                for k in range(NK):
                    nc.sync.dma_start(t[:, k, 0:256], src[k * P:(k + 1) * P,
                                                          c0:c0 + 256])
                    nc.sync.dma_start(t[:, k, 256:512], src[k * P:(k + 1) * P,
                                                            c0 + 256:c0 + 512])
                return t

            # ---- phase B: K^T over local s ----
            for h in range(2):                     # Wk dout halves
                wkh = stage_w(wk_d, h)
                for j in range(2):                 # local s 512-chunks
                    xs = stage_w(xtl_d, j)
                    for mm in range(4):
                        m = 4 * h + mm
                        ps = psum.tile([P, 512], f32, tag="ps512", bufs=2)
                        for k in range(NK):
                            nc.tensor.matmul(ps[:], wkh[:, k, mm * P:(mm + 1) * P],
                                             xs[:, k, :],
                                             start=(k == 0), stop=(k == NK - 1))
                        nc.vector.tensor_copy(kt_sb[:, m, 512 * j:512 * (j + 1)], ps[:])

            # ---- phase C: V over local s ----
            for n in range(2):                     # dv halves
                if n == 1:
                    # Wq is first needed in phase D; loading it here keeps the
                    # DMA queues free for phase B's operands at kernel start.
                    for k in range(NK):
                        nc.sync.dma_start(wq_t[:, k, :], wq_d[k * P:(k + 1) * P, :])
                    nc.sync.dma_start(mask_t[:, 0, :], mask_d[0])
                    nc.sync.dma_start(mask_t[:, 1, :], mask_d[1])
                    nc.sync.dma_start(ones_t[:], ones_d[:])
                wvh = stage_w(wv_d, n)
                for j in range(2):
                    xs = stage_w(xtl_d, j)
                    for lt4 in range(4):           # local 128-tiles in chunk j
                        lt = 4 * j + lt4
                        ps = psum.tile([P, 512], f32, tag="ps512", bufs=2)
                        for k in range(NK):
                            nc.tensor.matmul(ps[:], xs[:, k, lt4 * P:(lt4 + 1) * P],
                                             wvh[:, k, :],
                                             start=(k == 0), stop=(k == NK - 1))
                        nc.vector.tensor_copy(v_sb[:, lt, 512 * n:512 * (n + 1)], ps[:])

            # ---- phase D: per query chunk ----
            for i in range(QC):
                xq = stage_w(xt_d, i)
                qtp = pool.tile([P, NK, 512], f32r, tag="qtp", bufs=1)
                for m in range(NK):
                    ps = psum.tile([P, 512], f32, tag="ps512", bufs=2)
                    for k in range(NK):
                        nc.tensor.matmul(ps[:], wq_t[:, k, m * P:(m + 1) * P],
                                         xq[:, k, :],
                                         start=(k == 0), stop=(k == NK - 1))
                    nc.vector.tensor_copy(qtp[:, m, :], ps[:])

                nlt_all = 2 * i + 2
                pt = pool.tile([P, NK, 512], f32r, tag="pt", bufs=1)
                for lt in range(nlt_all):
                    ps = psum.tile([P, 512], f32, tag="ps512", bufs=2)
                    for m in range(NK):
                        nc.tensor.matmul(ps[:], kt_sb[:, m, lt * P:(lt + 1) * P],
                                         qtp[:, m, :],
                                         start=(m == 0), stop=(m == NK - 1))
                    if lt == 2 * i:
                        nc.vector.tensor_add(ps[:], ps[:], mask_t[:, 0, :])
                    elif lt == 2 * i + 1:
                        nc.vector.tensor_add(ps[:], ps[:], mask_t[:, 1, :])
                    nc.scalar.activation(pt[:, lt, :], ps[:],
                                         mybir.ActivationFunctionType.Exp,
                                         bias=0.0, scale=SCALE)

                for qb in range(4):
                    nlt = 2 * i + 1 if qb < 2 else 2 * i + 2
                    pso = psum.tile([P, D], f32, tag="psO", bufs=2)
                    pss = psum.tile([P, 2], f32, tag="psS", bufs=2)
                    for t_ in range(nlt):
                        lhs = pt[:, t_, qb * P:(qb + 1) * P]
                        st, sp = (t_ == 0), (t_ == nlt - 1)
                        nc.tensor.matmul(pso[:, 0:512], lhs, v_sb[:, t_, 0:512],
                                         start=st, stop=sp)
                        nc.tensor.matmul(pso[:, 512:1024], lhs, v_sb[:, t_, 512:1024],
                                         start=st, stop=sp)
                        nc.tensor.matmul(pss[:], lhs, ones_t[:], start=st, stop=sp)
                    osb = pool.tile([P, D + 1], f32, tag="osb", bufs=2)
                    nc.vector.tensor_copy(osb[:, 0:D], pso[:])
                    nc.vector.tensor_copy(osb[:, D:D + 1], pss[:, 0:1])
                    r0 = 512 * i + qb * P
                    nc.sync.dma_start(part_d[r0:r0 + P, :], osb[:])

    nc.finalize()
    return nc


def _get_program():
    global _prog
    if _prog is None:
        _prog = _build_program()
    return _prog


def kernel(x, Wq, Wk, Wv):
    from concourse.bass_utils import run_bass_kernel_spmd

    x = np.asarray(x, dtype=np.float32)
    Wq = np.ascontiguousarray(np.asarray(Wq, dtype=np.float32))
    Wk = np.ascontiguousarray(np.asarray(Wk, dtype=np.float32))
    Wv = np.ascontiguousarray(np.asarray(Wv, dtype=np.float32))

    ones = np.ones((P, 2), dtype=np.float32)
    sr = np.arange(P)[:, None]
    qr = np.arange(512)[None, :]
    masks = {}
    for p in (0, 1):
        m0 = np.where(128 * p + sr > qr, NEG, 0.0).astype(np.float32)
        m1 = np.where(128 * (2 + p) + sr > qr, NEG, 0.0).astype(np.float32)
        masks[p] = np.stack([m0, m1])

    in_maps = []
    for c in range(8):
        b, p = c // 2, c % 2
        xt = np.ascontiguousarray(x[b].T)                     # [D, T]
        xtv = xt.reshape(D, T // P, P)
        xtl = np.ascontiguousarray(
            xtv[:, p::2, :].reshape(D, T // 2))               # local s cols
        in_maps.append({
            "xt": xt, "xtl": xtl,
            "wq": Wq, "wk": Wk, "wv": Wv,
            "masks": masks[p], "ones": ones,
        })

    global _last_in_maps
    _last_in_maps = in_maps
    nc = _get_program()
    res = run_bass_kernel_spmd(nc, in_maps, list(range(8)))

    out = np.empty((B, T, D), dtype=np.float32)
    for b in range(B):
        p0 = res.results[2 * b]["part"]
        p1 = res.results[2 * b + 1]["part"]
        O = p0[:, :D] + p1[:, :D]
        d = p0[:, D] + p1[:, D]
        out[b] = O / d[:, None]
    return out


# revision 11
# speedup vs baseline: 1.1985x; 1.1985x over previous
"""Causal single-head attention (B=4, T=2048, D=1024, fp32) on 8 TRN2 NeuronCores.

Sharding: 2 cores per batch. Within a pair, keys/values are split by
interleaved 128-token tiles (core parity p takes s-tiles t with t%2==p), which
makes the program perfectly uniform across cores (SPMD): for every 512-wide
query chunk i, each core processes exactly 2i+2 local key tiles. Each core
computes an unnormalized partial attention output plus softmax denominators
for ALL queries of its batch; the host merges the two partials per batch
(add, then divide) while unsharding.

All matmuls run as fp32r (TF32-like, ~1.5e-4 rel err, full PE rate at N>=256).
Softmax is computed without max-subtraction: logits = scores/32 are bounded by
~|q||k|/32 ~ +-8 for this distribution, far from fp32 exp overflow.
"""
import numpy as np

B, T, D = 4, 2048, 1024
P = 128
NK = D // P          # 8 contraction tiles
QC = T // 512        # 4 query chunks of 512
NEG = -1e30
SCALE = 1.0 / 32.0   # 1/sqrt(D)

_prog = None
_last_in_maps = None


def _build_program():
    import concourse.bacc as bacc
    import concourse.mybir as mybir
    import concourse.tile as tile

    f32 = mybir.dt.float32
    f32r = mybir.dt.float32r

    nc = bacc.Bacc()
    xt_d = nc.declare_dram_parameter("xt", [D, T], f32r, isOutput=False)
    xtl_d = nc.declare_dram_parameter("xtl", [D, T // 2], f32r, isOutput=False)
    wq_d = nc.declare_dram_parameter("wq", [D, D], f32r, isOutput=False)
    wk_d = nc.declare_dram_parameter("wk", [D, D], f32r, isOutput=False)
    wv_d = nc.declare_dram_parameter("wv", [D, D], f32r, isOutput=False)
    mask_d = nc.declare_dram_parameter("masks", [2, P, 512], f32, isOutput=False)
    ones_d = nc.declare_dram_parameter("ones", [P, 2], f32r, isOutput=False)
    part_d = nc.declare_dram_parameter("part", [T, D + 1], f32, isOutput=True)

    with tile.TileContext(nc) as tc:
        with tc.tile_pool(name="sbuf", bufs=1) as pool, \
             tc.tile_pool(name="psum", bufs=1, space="PSUM") as psum:

            # ---- long-lived tiles ----
            wq_t = pool.tile([P, NK, D], f32r, tag="wq")       # Wq, pinned
            kt_sb = pool.tile([P, NK, T // 2], f32r, tag="kt")  # K^T, local s
            v_sb = pool.tile([P, NK, D], f32r, tag="v")         # V, local s tiles
            mask_t = pool.tile([P, 2, 512], f32, tag="mask")
            ones_t = pool.tile([P, 2], f32r, tag="ones")

            def stage_w(src, h):
                """load [128, NK, 512] = src[:, 512h:512h+512] by k-tiles,
                two DMAs per k-tile so the first consumer waits ~half as long"""
                t = pool.tile([P, NK, 512], f32r, tag="stage", bufs=4)
                c0 = 512 * h
                for k in range(NK):
                    nc.sync.dma_start(t[:, k, :], src[k * P:(k + 1) * P,
                                                      c0:c0 + 512])
                return t

            # ---- phase B: K^T over local s ----
            for h in range(2):                     # Wk dout halves
                wkh = stage_w(wk_d, h)
                for j in range(2):                 # local s 512-chunks
                    xs = stage_w(xtl_d, j)
                    for mm in range(4):
                        m = 4 * h + mm
                        ps = psum.tile([P, 512], f32, tag="ps512", bufs=2)
                        for k in range(NK):
                            nc.tensor.matmul(ps[:], wkh[:, k, mm * P:(mm + 1) * P],
                                             xs[:, k, :],
                                             start=(k == 0), stop=(k == NK - 1))
                        nc.vector.tensor_copy(kt_sb[:, m, 512 * j:512 * (j + 1)], ps[:])

            # ---- phase C: V over local s ----
            for n in range(2):                     # dv halves
                if n == 1:
                    # Wq is first needed in phase D; loading it here keeps the
                    # DMA queues free for phase B's operands at kernel start.
                    for k in range(NK):
                        nc.sync.dma_start(wq_t[:, k, :], wq_d[k * P:(k + 1) * P, :])
                    nc.sync.dma_start(mask_t[:, 0, :], mask_d[0])
                    nc.sync.dma_start(mask_t[:, 1, :], mask_d[1])
                    nc.sync.dma_start(ones_t[:], ones_d[:])
                wvh = stage_w(wv_d, n)
                for j in range(2):
                    xs = stage_w(xtl_d, j)
                    for lt4 in range(4):           # local 128-tiles in chunk j
                        lt = 4 * j + lt4
                        ps = psum.tile([P, 512], f32, tag="ps512", bufs=2)
                        for k in range(NK):
                            nc.tensor.matmul(ps[:], xs[:, k, lt4 * P:(lt4 + 1) * P],
                                             wvh[:, k, :],
                                             start=(k == 0), stop=(k == NK - 1))
                        nc.vector.tensor_copy(v_sb[:, lt, 512 * n:512 * (n + 1)], ps[:])

            # ---- phase D: per query chunk ----
            for i in range(QC):
                xq = stage_w(xt_d, i)
                qtp = pool.tile([P, NK, 512], f32r, tag="qtp", bufs=1)
                for m in range(NK):
                    ps = psum.tile([P, 512], f32, tag="ps512", bufs=2)
                    for k in range(NK):
                        nc.tensor.matmul(ps[:], wq_t[:, k, m * P:(m + 1) * P],
                                         xq[:, k, :],
                                         start=(k == 0), stop=(k == NK - 1))
                    nc.vector.tensor_copy(qtp[:, m, :], ps[:])

                nlt_all = 2 * i + 2
                pt = pool.tile([P, NK, 512], f32r, tag="pt", bufs=1)
                for lt in range(nlt_all):
                    ps = psum.tile([P, 512], f32, tag="ps512", bufs=2)
                    for m in range(NK):
                        nc.tensor.matmul(ps[:], kt_sb[:, m, lt * P:(lt + 1) * P],
                                         qtp[:, m, :],
                                         start=(m == 0), stop=(m == NK - 1))
                    if lt == 2 * i:
                        nc.vector.tensor_add(ps[:], ps[:], mask_t[:, 0, :])
                    elif lt == 2 * i + 1:
                        nc.vector.tensor_add(ps[:], ps[:], mask_t[:, 1, :])
                    nc.scalar.activation(pt[:, lt, :], ps[:],
                                         mybir.ActivationFunctionType.Exp,
                                         bias=0.0, scale=SCALE)

                for qb in range(4):
                    nlt = 2 * i + 1 if qb < 2 else 2 * i + 2
                    pso = psum.tile([P, D], f32, tag="psO", bufs=2)
                    pss = psum.tile([P, 2], f32, tag="psS", bufs=2)
                    for t_ in range(nlt):
                        lhs = pt[:, t_, qb * P:(qb + 1) * P]
                        st, sp = (t_ == 0), (t_ == nlt - 1)
                        nc.tensor.matmul(pso[:, 0:512], lhs, v_sb[:, t_, 0:512],
                                         start=st, stop=sp)
                        nc.tensor.matmul(pso[:, 512:1024], lhs, v_sb[:, t_, 512:1024],
                                         start=st, stop=sp)
                        nc.tensor.matmul(pss[:], lhs, ones_t[:], start=st, stop=sp)
                    osb = pool.tile([P, D + 1], f32, tag="osb", bufs=2)
                    nc.vector.tensor_copy(osb[:, 0:D], pso[:])
                    nc.vector.tensor_copy(osb[:, D:D + 1], pss[:, 0:1])
                    r0 = 512 * i + qb * P
                    nc.sync.dma_start(part_d[r0:r0 + P, :], osb[:])

    nc.finalize()
    return nc


def _get_program():
    global _prog
    if _prog is None:
        _prog = _build_program()
    return _prog


def kernel(x, Wq, Wk, Wv):
    from concourse.bass_utils import run_bass_kernel_spmd

    x = np.asarray(x, dtype=np.float32)
    Wq = np.ascontiguousarray(np.asarray(Wq, dtype=np.float32))
    Wk = np.ascontiguousarray(np.asarray(Wk, dtype=np.float32))
    Wv = np.ascontiguousarray(np.asarray(Wv, dtype=np.float32))

    ones = np.ones((P, 2), dtype=np.float32)
    sr = np.arange(P)[:, None]
    qr = np.arange(512)[None, :]
    masks = {}
    for p in (0, 1):
        m0 = np.where(128 * p + sr > qr, NEG, 0.0).astype(np.float32)
        m1 = np.where(128 * (2 + p) + sr > qr, NEG, 0.0).astype(np.float32)
        masks[p] = np.stack([m0, m1])

    in_maps = []
    for c in range(8):
        b, p = c // 2, c % 2
        xt = np.ascontiguousarray(x[b].T)                     # [D, T]
        xtv = xt.reshape(D, T // P, P)
        xtl = np.ascontiguousarray(
            xtv[:, p::2, :].reshape(D, T // 2))               # local s cols
        in_maps.append({
            "xt": xt, "xtl": xtl,
            "wq": Wq, "wk": Wk, "wv": Wv,
            "masks": masks[p], "ones": ones,
        })

    global _last_in_maps
    _last_in_maps = in_maps
    nc = _get_program()
    res = run_bass_kernel_spmd(nc, in_maps, list(range(8)))

    out = np.empty((B, T, D), dtype=np.float32)
    for b in range(B):
        p0 = res.results[2 * b]["part"]
        p1 = res.results[2 * b + 1]["part"]
        O = p0[:, :D] + p1[:, :D]
        d = p0[:, D] + p1[:, D]
        out[b] = O / d[:, None]
    return out


# revision 13
# speedup vs baseline: 1.2460x; 1.0396x over previous
"""Causal single-head attention (B=4, T=2048, D=1024, fp32) on 8 TRN2 NeuronCores.

Sharding: 2 cores per batch. Within a pair, keys/values are split by
interleaved 128-token tiles (core parity p takes s-tiles t with t%2==p), which
makes the program perfectly uniform across cores (SPMD): for every 512-wide
query chunk i, each core processes exactly 2i+2 local key tiles. Each core
computes an unnormalized partial attention output plus softmax denominators
for ALL queries of its batch; the host merges the two partials per batch
(add, then divide) while unsharding.

All matmuls run as fp32r (TF32-like, ~1.5e-4 rel err, full PE rate at N>=256).
Softmax is computed without max-subtraction: logits = scores/32 are bounded by
~|q||k|/32 ~ +-8 for this distribution, far from fp32 exp overflow.
"""
import numpy as np

B, T, D = 4, 2048, 1024
P = 128
NK = D // P          # 8 contraction tiles
QC = T // 512        # 4 query chunks of 512
NEG = -1e30
SCALE = 1.0 / 32.0   # 1/sqrt(D)

_prog = None
_last_in_maps = None


def _build_program():
    import concourse.bacc as bacc
    import concourse.mybir as mybir
    import concourse.tile as tile

    f32 = mybir.dt.float32
    f32r = mybir.dt.float32r

    nc = bacc.Bacc()
    xt_d = nc.declare_dram_parameter("xt", [D, T], f32r, isOutput=False)
    xtl_d = nc.declare_dram_parameter("xtl", [D, T // 2], f32r, isOutput=False)
    wq_d = nc.declare_dram_parameter("wq", [D, D], f32r, isOutput=False)
    wk_d = nc.declare_dram_parameter("wk", [D, D], f32r, isOutput=False)
    wv_d = nc.declare_dram_parameter("wv", [D, D], f32r, isOutput=False)
    mask_d = nc.declare_dram_parameter("masks", [2, P, 512], f32, isOutput=False)
    ones_d = nc.declare_dram_parameter("ones", [P, 2], f32r, isOutput=False)
    part_d = nc.declare_dram_parameter("part", [T, D + 1], f32, isOutput=True)

    with tile.TileContext(nc) as tc:
        with tc.tile_pool(name="sbuf", bufs=1) as pool, \
             tc.tile_pool(name="psum", bufs=1, space="PSUM") as psum:

            # ---- long-lived tiles ----
            wq_t = pool.tile([P, NK, D], f32r, tag="wq")       # Wq, pinned
            kt_sb = pool.tile([P, NK, T // 2], f32r, tag="kt")  # K^T, local s
            v_sb = pool.tile([P, NK, D], f32r, tag="v")         # V, local s tiles
            mask_t = pool.tile([P, 2, 512], f32, tag="mask")
            ones_t = pool.tile([P, 2], f32r, tag="ones")

            # Wq is first needed in phase D; dribbling one k-tile of it after
            # each B/C stage load keeps the DMA queues free for the operands
            # the PE is actually waiting on.
            wq_next = [0]

            def stage_w(src, h, wq_dribble=True):
                """load [128, NK, 512] = src[:, 512h:512h+512] by k-tiles"""
                t = pool.tile([P, NK, 512], f32r, tag="stage", bufs=4)
                c0 = 512 * h
                for k in range(NK):
                    nc.sync.dma_start(t[:, k, :], src[k * P:(k + 1) * P,
                                                      c0:c0 + 512])
                if wq_dribble and wq_next[0] < NK:
                    k = wq_next[0]
                    wq_next[0] += 1
                    nc.sync.dma_start(wq_t[:, k, :], wq_d[k * P:(k + 1) * P, :])
                    if k == 0:
                        nc.sync.dma_start(mask_t[:, 0, :], mask_d[0])
                        nc.sync.dma_start(mask_t[:, 1, :], mask_d[1])
                        nc.sync.dma_start(ones_t[:], ones_d[:])
                return t

            # ---- phase B: K^T over local s ----
            for h in range(2):                     # Wk dout halves
                wkh = stage_w(wk_d, h, wq_dribble=(h > 0))
                for j in range(2):                 # local s 512-chunks
                    xs = stage_w(xtl_d, j, wq_dribble=(h + j > 0))
                    for mm in range(4):
                        m = 4 * h + mm
                        ps = psum.tile([P, 512], f32, tag="ps512", bufs=2)
                        for k in range(NK):
                            nc.tensor.matmul(ps[:], wkh[:, k, mm * P:(mm + 1) * P],
                                             xs[:, k, :],
                                             start=(k == 0), stop=(k == NK - 1))
                        nc.vector.tensor_copy(kt_sb[:, m, 512 * j:512 * (j + 1)], ps[:])

            # ---- phase C: V over local s ----
            for n in range(2):                     # dv halves
                wvh = stage_w(wv_d, n)
                for j in range(2):
                    xs = stage_w(xtl_d, j)
                    for lt4 in range(4):           # local 128-tiles in chunk j
                        lt = 4 * j + lt4
                        ps = psum.tile([P, 512], f32, tag="ps512", bufs=2)
                        for k in range(NK):
                            nc.tensor.matmul(ps[:], xs[:, k, lt4 * P:(lt4 + 1) * P],
                                             wvh[:, k, :],
                                             start=(k == 0), stop=(k == NK - 1))
                        nc.vector.tensor_copy(v_sb[:, lt, 512 * n:512 * (n + 1)], ps[:])

            # ---- phase D: per query chunk ----
            for i in range(QC):
                xq = stage_w(xt_d, i)
                qtp = pool.tile([P, NK, 512], f32r, tag="qtp", bufs=1)
                for m in range(NK):
                    ps = psum.tile([P, 512], f32, tag="ps512", bufs=2)
                    for k in range(NK):
                        nc.tensor.matmul(ps[:], wq_t[:, k, m * P:(m + 1) * P],
                                         xq[:, k, :],
                                         start=(k == 0), stop=(k == NK - 1))
                    nc.vector.tensor_copy(qtp[:, m, :], ps[:])

                nlt_all = 2 * i + 2
                pt = pool.tile([P, NK, 512], f32r, tag="pt", bufs=1)
                for lt in range(nlt_all):
                    ps = psum.tile([P, 512], f32, tag="ps512", bufs=2)
                    for m in range(NK):
                        nc.tensor.matmul(ps[:], kt_sb[:, m, lt * P:(lt + 1) * P],
                                         qtp[:, m, :],
                                         start=(m == 0), stop=(m == NK - 1))
                    if lt == 2 * i:
                        nc.vector.tensor_add(ps[:], ps[:], mask_t[:, 0, :])
                    elif lt == 2 * i + 1:
                        nc.vector.tensor_add(ps[:], ps[:], mask_t[:, 1, :])
                    nc.scalar.activation(pt[:, lt, :], ps[:],
                                         mybir.ActivationFunctionType.Exp,
                                         bias=0.0, scale=SCALE)

                for qb in range(4):
                    nlt = 2 * i + 1 if qb < 2 else 2 * i + 2
                    pso = psum.tile([P, D], f32, tag="psO", bufs=2)
                    pss = psum.tile([P, 2], f32, tag="psS", bufs=2)
                    for t_ in range(nlt):
                        lhs = pt[:, t_, qb * P:(qb + 1) * P]
                        st, sp = (t_ == 0), (t_ == nlt - 1)
                        nc.tensor.matmul(pso[:, 0:512], lhs, v_sb[:, t_, 0:512],
                                         start=st, stop=sp)
                        nc.tensor.matmul(pso[:, 512:1024], lhs, v_sb[:, t_, 512:1024],
                                         start=st, stop=sp)
                        nc.tensor.matmul(pss[:], lhs, ones_t[:], start=st, stop=sp)
                    osb = pool.tile([P, D + 1], f32, tag="osb", bufs=2)
                    nc.vector.tensor_copy(osb[:, 0:D], pso[:])
                    nc.vector.tensor_copy(osb[:, D:D + 1], pss[:, 0:1])
                    r0 = 512 * i + qb * P
                    nc.sync.dma_start(part_d[r0:r0 + P, :], osb[:])

    nc.finalize()
    return nc


def _get_program():
    global _prog
    if _prog is None:
        _prog = _build_program()
    return _prog


def kernel(x, Wq, Wk, Wv):
    from concourse.bass_utils import run_bass_kernel_spmd

    x = np.asarray(x, dtype=np.float32)
    Wq = np.ascontiguousarray(np.asarray(Wq, dtype=np.float32))
    Wk = np.ascontiguousarray(np.asarray(Wk, dtype=np.float32))
    Wv = np.ascontiguousarray(np.asarray(Wv, dtype=np.float32))

    ones = np.ones((P, 2), dtype=np.float32)
    sr = np.arange(P)[:, None]
    qr = np.arange(512)[None, :]
    masks = {}
    for p in (0, 1):
        m0 = np.where(128 * p + sr > qr, NEG, 0.0).astype(np.float32)
        m1 = np.where(128 * (2 + p) + sr > qr, NEG, 0.0).astype(np.float32)
        masks[p] = np.stack([m0, m1])

    in_maps = []
    for c in range(8):
        b, p = c // 2, c % 2
        xt = np.ascontiguousarray(x[b].T)                     # [D, T]
        xtv = xt.reshape(D, T // P, P)
        xtl = np.ascontiguousarray(
            xtv[:, p::2, :].reshape(D, T // 2))               # local s cols
        in_maps.append({
            "xt": xt, "xtl": xtl,
            "wq": Wq, "wk": Wk, "wv": Wv,
            "masks": masks[p], "ones": ones,
        })

    global _last_in_maps
    _last_in_maps = in_maps
    nc = _get_program()
    res = run_bass_kernel_spmd(nc, in_maps, list(range(8)))

    out = np.empty((B, T, D), dtype=np.float32)
    for b in range(B):
        p0 = res.results[2 * b]["part"]
        p1 = res.results[2 * b + 1]["part"]
        O = p0[:, :D] + p1[:, :D]
        d = p0[:, D] + p1[:, D]
        out[b] = O / d[:, None]
    return out
